# revision 10
# baseline (speedup 1.0000x reference)
"""Trainium2 Bass kernel for nn_CustomLoss_67989332295833 (v2).

loss = mean_b[ -t_b * ( sum_j p*neigh*logp + (sum_j logp + log(1-p))/N ) ]
with p = sigmoid(x), neigh_j = p_{j-1} + p_{j+1} (zero boundaries).

Reformulation used here (per row, m_j := softplus(-x_j) = -ln p_j):
  sigma_j = m_j + m_{j+1} = -ln w_j          (w_j = p_j * p_{j+1})
  h-term  = -sum_{j<N-1} w_j ln w_j = sum_j sigma_j * exp(-sigma_j)
  sum_j ln(1-p_j) = sum_j ln p_j - sum_j x_j  (exact identity)
  loss*B  = sum_r t_r * ( A'_r + (2*S_r + rx_r)/N )
  A'_r = sum_j sigma_j w_j,  S_r = sum_j m_j,  rx_r = sum_j x_j

Engine mapping (the whole point of this formulation):
  ACT: e = exp(-x) [fp16], m = ln(1+e) [fp16, free bias=1] -- BOTH functions
       live in the natural_log_exp_and_others table set: one ACT table load,
       zero switching, zero ordering constraints (vs sigmoid/ln which live in
       different sets and cost ~2.7us per switch).
  DVE: sc  = (m[1:] + K) + m[:-1]            scalar_tensor_tensor
       i16 = int16(max(sc*C1, 0))            tensor_scalar (4x mode)
       w   = i16.bitcast(fp16)               = 2^(...) exp bit trick: i16 is
             round(C1*sigma + C2) which, read as fp16 bits, is exp(-sigma)
             to ~0.5% (Schraudolph constant folded so K = C2/C1 and the
             max(.,0) clamps the fp16-subnormal underflow region)
       u   = (sc - K) * w, accum_out=A'[:,g] scalar_tensor_tensor (2x mode)
  PE : SU_m += t^T @ m-chunks (fp16), SU_x += t^T @ x-chunks (float32r)
       t-weighted aggregates of S_r and rx_r; host applies the 1/N scales.
  Optional: for groups in `offload`, e comes from the same exp2 bit trick on
       DVE (tensor_scalar f32->int16) instead of ACT, trading ACT time for
       DVE time to balance the two engines.

Sharding: pure data-parallel over batch, 1024 rows/core on 8 cores. Outputs
per core: loss_rows[P,G] = t*A' per row, su[2,512] = (m-bucket, x-bucket).
Host: loss = (sum(loss_rows) + (2*sum(su[0]) + sum(su[1]))/N) / B.
"""

from contextlib import ExitStack

import numpy as np

import concourse.bacc as bacc
import concourse.bass as bass
import concourse.mybir as mybir
import concourse.tile as tile
from concourse.bass_utils import run_bass_kernel_spmd

B, N = 8192, 4096
NCORES = 8
ROWS = B // NCORES          # rows per core
P = 128                     # SBUF partitions
G = ROWS // P               # 128-row groups per core
F32 = mybir.dt.float32
F32R = mybir.dt.float32r
F16 = mybir.dt.float16
I16 = mybir.dt.int16

# exp2 bit-trick constants (fp16 layout: exp bias 15 at bit 10).
# bits = round(1024*(15 - kappa - sigma*log2(e))) read as fp16 ~= exp(-sigma).
# kappa = 0.0573 zeroes the mean of the linear-mantissa curve error (verified
# in float simulation of this exact pipeline: rel err ~4e-5 vs f64).
KAPPA = 0.0573
C1 = float(-1024.0 * np.log2(np.e))
C2 = float(1024.0 * (15.0 - KAPPA))
K = C2 / C1                 # ~ -10.394; sc = sigma + K so i16 = round(sc*C1)


def build_kernel(
    offload=(),
    loop_M=None,
    bufs_x=3,
    bufs_e=2,
    bufs_m=3,
    bufs_sc=2,
    bufs_i=2,
    bufs_u=2,
    split_first_dma=True,
):
    offload = set(offload)
    nc = bacc.Bacc(
        "TRN2",
        target_bir_lowering=False,
        debug=False,
        enable_asserts=False,
        num_devices=NCORES,
    )
    # x is declared float32r (bit-identical to f32) so the PE x-bucket can
    # consume it at 1 cycle/row; ACT/DVE read it through a .bitcast(F32) view.
    x_d = nc.dram_tensor("x", [G, P, N], F32R, kind="ExternalInput")
    t_d = nc.dram_tensor("t", [G, P, 1], F32R, kind="ExternalInput")
    out_d = nc.dram_tensor("loss_rows", [P, G], F32, kind="ExternalOutput")
    su_d = nc.dram_tensor("su", [2, 512], F32, kind="ExternalOutput")

    CH = N // 512  # PE column chunks per group

    Exp = mybir.ActivationFunctionType.Exp
    Ln = mybir.ActivationFunctionType.Ln
    add = mybir.AluOpType.add
    mult = mybir.AluOpType.mult
    amax = mybir.AluOpType.max

    with tile.TileContext(nc) as tc, ExitStack() as ctx:
        x = x_d.ap()

        xpool = ctx.enter_context(tc.tile_pool(name="xp", bufs=bufs_x))
        epool = ctx.enter_context(tc.tile_pool(name="ep", bufs=bufs_e))
        mpool = ctx.enter_context(tc.tile_pool(name="mp", bufs=bufs_m))
        scpool = ctx.enter_context(tc.tile_pool(name="scp", bufs=bufs_sc))
        ipool = ctx.enter_context(tc.tile_pool(name="ip", bufs=bufs_i))
        upool = ctx.enter_context(tc.tile_pool(name="up", bufs=bufs_u))
        small = ctx.enter_context(tc.tile_pool(name="small", bufs=1))
        psum = ctx.enter_context(tc.tile_pool(name="psum", bufs=1, space="PSUM"))

        # Pin the ACT table to natural_log_exp_and_others (id 6), which holds
        # BOTH exp and ln. Without this, the auto-inserter greedily picks
        # exp_and_others for Exp and natural_log for Ln and thrashes (10
        # loads, ~2.7us each); with the manual load it inserts none.
        nc.scalar.add_instruction(
            mybir.InstLoadActFuncSet(name="manual_atl", act_func_set_id=6)
        )

        loop_cm = tc.For_i(0, loop_M, 1) if loop_M else None
        if loop_cm is not None:
            ctx.enter_context(loop_cm)

        # targets: one strided SWDGE DMA (separate queue from the x stream).
        # Tile is f32r so it can be the x-bucket lhsT; ttf = f32 view for DVE.
        ttr = small.tile([P, G], F32R, tag="ttr")
        t_src = bass.AP(tensor=t_d, offset=0, ap=[[1, P], [P, G]])
        nc.gpsimd.dma_start(out=ttr, in_=t_src)
        ttf = ttr.bitcast(F32)
        tb = small.tile([P, G], F16, tag="tb")      # fp16 lhsT for m-bucket
        nc.vector.tensor_copy(tb, ttf)

        AH = small.tile([P, G], F32, tag="AH")      # per-row A' accumulators
        SUM = psum.tile([1, 512], F32, tag="SUM")   # t-weighted sum of m
        SUX = psum.tile([1, 512], F32, tag="SUX")   # t-weighted sum of x

        n_m_mm = 0
        n_x_mm = 0
        for g in range(G):
            # ---- load x (f32r tile; xf = f32 view for ACT/DVE) ----
            xt = xpool.tile([P, N], F32R, tag="xt")
            xf = xt.bitcast(F32)
            if g == 0 and split_first_dma:
                H = N // 2
                nc.sync.dma_start(out=xt[:, 0:H], in_=x[g][:, 0:H])
                nc.sync.dma_start(out=xt[:, H:N], in_=x[g][:, H:N])
            else:
                nc.sync.dma_start(out=xt, in_=x[g])

            # ---- e = exp(-x) ----
            if g in offload:
                # DVE exp2 bit trick: i16 = round(C1*x + C2) -> fp16 bits
                iet = epool.tile([P, N], I16, tag="iet")
                nc.vector.tensor_scalar(iet, xf, C1, C2, mult, add)
                et = iet.bitcast(F16)
            else:
                et = epool.tile([P, N], F16, tag="et")
                nc.scalar.activation(out=et, in_=xf, func=Exp, scale=-1.0)

            # ---- m = ln(1 + e) = softplus(-x) ----
            mt = mpool.tile([P, N], F16, tag="mt")
            nc.scalar.activation(out=mt, in_=et, func=Ln, bias=1.0)

            # ---- sc = (m[1:] + K) + m[:-1]; sc[N-1] = K (sigma=0 -> u=0) ----
            sct = scpool.tile([P, N], F16, tag="sct")
            nc.vector.memset(sct[:, N - 1 : N], K)
            nc.vector.scalar_tensor_tensor(
                out=sct[:, 0 : N - 1],
                in0=mt[:, 1:N],
                scalar=K,
                in1=mt[:, 0 : N - 1],
                op0=add,
                op1=add,
            )

            # ---- i16 = int16(max(sc*C1, 0)); w = bits as fp16 ----
            i16t = ipool.tile([P, N], I16, tag="i16t")
            nc.vector.tensor_scalar(i16t, sct, C1, 0.0, mult, amax)
            wt = i16t.bitcast(F16)

            # ---- u = (sc - K) * w = sigma*exp(-sigma); A'[:,g] = sum_j u ----
            ut = upool.tile([P, N], F16, tag="ut")
            nc.vector.scalar_tensor_tensor(
                out=ut,
                in0=sct,
                scalar=-K,
                in1=wt,
                op0=add,
                op1=mult,
                accum_out=AH[:, g : g + 1],
            )

            # ---- PE buckets: SUM += t^T m, SUX += t^T x ----
            for c in range(CH):
                nc.tensor.matmul(
                    SUM,
                    tb[:, g : g + 1],
                    mt[:, c * 512 : (c + 1) * 512],
                    start=(n_m_mm == 0),
                    stop=(n_m_mm == G * CH - 1),
                )
                n_m_mm += 1
            for c in range(CH):
                nc.tensor.matmul(
                    SUX,
                    ttr[:, g : g + 1],
                    xt[:, c * 512 : (c + 1) * 512],
                    start=(n_x_mm == 0),
                    stop=(n_x_mm == G * CH - 1),
                )
                n_x_mm += 1

        # ---- outputs: loss_rows = t * A', su = [SUM; SUX] ----
        L = small.tile([P, G], F32, tag="L")
        nc.vector.tensor_mul(L, AH, ttf)
        nc.sync.dma_start(out=out_d.ap(), in_=L)

        susm = small.tile([1, 512], F32, tag="susm")
        nc.vector.tensor_copy(susm, SUM)
        nc.sync.dma_start(out=su_d.ap()[0:1, :], in_=susm)
        susx = small.tile([1, 512], F32, tag="susx")
        nc.vector.tensor_copy(susx, SUX)
        nc.sync.dma_start(out=su_d.ap()[1:2, :], in_=susx)

    nc.finalize()
    return nc


_NC_CACHE = {}

# Groups whose exp(-x) runs on DVE instead of ACT (balance the engines).
BEST_OFFLOAD = ()


def _get_nc():
    if "nc" not in _NC_CACHE:
        _NC_CACHE["nc"] = build_kernel(offload=BEST_OFFLOAD)
    return _NC_CACHE["nc"]


def run_sharded(inputs, targets, trace=False, nc=None):
    if nc is None:
        nc = _get_nc()
    in_maps = []
    for c in range(NCORES):
        xs = np.ascontiguousarray(
            inputs[c * ROWS : (c + 1) * ROWS].reshape(G, P, N), dtype=np.float32
        )
        ts = np.ascontiguousarray(
            targets[c * ROWS : (c + 1) * ROWS].reshape(G, P, 1), dtype=np.float32
        )
        in_maps.append({"x": xs, "t": ts})
    res = run_bass_kernel_spmd(
        nc, in_maps, core_ids=list(range(NCORES)), trace=trace
    )
    total = 0.0
    for r in res.results:
        total += r["loss_rows"].astype(np.float64).sum()
        su = r["su"].astype(np.float64)
        total += (2.0 * su[0].sum() + su[1].sum()) / N
    loss = np.float32(total / B)
    return loss, res


def kernel(inputs, targets):
    inputs = np.asarray(inputs, dtype=np.float32)
    targets = np.asarray(targets, dtype=np.float32)
    loss, _ = run_sharded(inputs, targets, trace=False)
    return loss


# revision 17
# speedup vs baseline: 1.3332x; 1.3332x over previous
"""Trainium2 Bass kernel for nn_CustomLoss_67989332295833 (v2).

loss = mean_b[ -t_b * ( sum_j p*neigh*logp + (sum_j logp + log(1-p))/N ) ]
with p = sigmoid(x), neigh_j = p_{j-1} + p_{j+1} (zero boundaries).

Reformulation used here (per row, m_j := softplus(-x_j) = -ln p_j):
  sigma_j = m_j + m_{j+1} = -ln w_j          (w_j = p_j * p_{j+1})
  h-term  = -sum_{j<N-1} w_j ln w_j = sum_j sigma_j * exp(-sigma_j)
  sum_j ln(1-p_j) = sum_j ln p_j - sum_j x_j  (exact identity)
  loss*B  = sum_r t_r * ( A'_r + (2*S_r + rx_r)/N )
  A'_r = sum_j sigma_j w_j,  S_r = sum_j m_j,  rx_r = sum_j x_j

Engine mapping (the whole point of this formulation):
  ACT: e = exp(-x) [fp16], m = ln(1+e) [fp16, free bias=1] -- BOTH functions
       live in the natural_log_exp_and_others table set: one ACT table load,
       zero switching, zero ordering constraints (vs sigmoid/ln which live in
       different sets and cost ~2.7us per switch).
  DVE: sc  = (m[1:] + K) + m[:-1]            scalar_tensor_tensor
       i16 = int16(max(sc*C1, 0))            tensor_scalar (4x mode)
       w   = i16.bitcast(fp16)               = 2^(...) exp bit trick: i16 is
             round(C1*sigma + C2) which, read as fp16 bits, is exp(-sigma)
             to ~0.5% (Schraudolph constant folded so K = C2/C1 and the
             max(.,0) clamps the fp16-subnormal underflow region)
       u   = (sc - K) * w, accum_out=A'[:,g] scalar_tensor_tensor (2x mode)
  PE : SU_m += t^T @ m-chunks (fp16), SU_x += t^T @ x-chunks (float32r)
       t-weighted aggregates of S_r and rx_r; host applies the 1/N scales.
  Optional: for groups in `offload`, e comes from the same exp2 bit trick on
       DVE (tensor_scalar f32->int16) instead of ACT, trading ACT time for
       DVE time to balance the two engines.

Sharding: pure data-parallel over batch, 1024 rows/core on 8 cores. Outputs
per core: loss_rows[P,G] = t*A' per row, su[2,512] = (m-bucket, x-bucket).
Host: loss = (sum(loss_rows) + (2*sum(su[0]) + sum(su[1]))/N) / B.
"""

from contextlib import ExitStack

import numpy as np

import concourse.bacc as bacc
import concourse.bass as bass
import concourse.mybir as mybir
import concourse.tile as tile
from concourse.bass_utils import run_bass_kernel_spmd

B, N = 8192, 4096
NCORES = 8
ROWS = B // NCORES          # rows per core
P = 128                     # SBUF partitions
G = ROWS // P               # 128-row groups per core
F32 = mybir.dt.float32
F32R = mybir.dt.float32r
F16 = mybir.dt.float16
I16 = mybir.dt.int16

# exp2 bit-trick constants (fp16 layout: exp bias 15 at bit 10).
# bits = round(1024*(15 - kappa - sigma*log2(e))) read as fp16 ~= exp(-sigma).
# kappa = 0.0573 zeroes the mean of the linear-mantissa curve error (verified
# in float simulation of this exact pipeline: rel err ~4e-5 vs f64).
KAPPA = 0.0573
C1 = float(-1024.0 * np.log2(np.e))
C2 = float(1024.0 * (15.0 - KAPPA))
K = C2 / C1                 # ~ -10.394; sc = sigma + K so i16 = round(sc*C1)


def build_kernel(
    offload=(),
    loop_M=None,
    bufs_x=3,
    bufs_e=2,
    bufs_m=3,
    bufs_sc=2,
    bufs_i=2,
    bufs_u=2,
    split_first_dma=True,
):
    offload = set(offload)
    nc = bacc.Bacc(
        "TRN2",
        target_bir_lowering=False,
        debug=False,
        enable_asserts=False,
        num_devices=NCORES,
    )
    # x is declared float32r (bit-identical to f32) so the PE x-bucket can
    # consume it at 1 cycle/row; ACT/DVE read it through a .bitcast(F32) view.
    x_d = nc.dram_tensor("x", [G, P, N], F32R, kind="ExternalInput")
    t_d = nc.dram_tensor("t", [G, P, 1], F32R, kind="ExternalInput")
    out_d = nc.dram_tensor("loss_rows", [P, G], F32, kind="ExternalOutput")
    su_d = nc.dram_tensor("su", [1, 512], F32, kind="ExternalOutput")

    CH = N // 512  # PE column chunks per group

    Exp = mybir.ActivationFunctionType.Exp
    Ln = mybir.ActivationFunctionType.Ln
    add = mybir.AluOpType.add
    mult = mybir.AluOpType.mult
    amax = mybir.AluOpType.max

    with tile.TileContext(nc) as tc, ExitStack() as ctx:
        x = x_d.ap()

        xpool = ctx.enter_context(tc.tile_pool(name="xp", bufs=bufs_x))
        epool = ctx.enter_context(tc.tile_pool(name="ep", bufs=bufs_e))
        mpool = ctx.enter_context(tc.tile_pool(name="mp", bufs=bufs_m))
        scpool = ctx.enter_context(tc.tile_pool(name="scp", bufs=bufs_sc))
        ipool = ctx.enter_context(tc.tile_pool(name="ip", bufs=bufs_i))
        upool = ctx.enter_context(tc.tile_pool(name="up", bufs=bufs_u))
        small = ctx.enter_context(tc.tile_pool(name="small", bufs=1))
        psum = ctx.enter_context(tc.tile_pool(name="psum", bufs=1, space="PSUM"))

        # Pin the ACT table to natural_log_exp_and_others (id 6), which holds
        # BOTH exp and ln. Without this, the auto-inserter greedily picks
        # exp_and_others for Exp and natural_log for Ln and thrashes (10
        # loads, ~2.7us each); with the manual load it inserts none.
        nc.scalar.add_instruction(
            mybir.InstLoadActFuncSet(name="manual_atl", act_func_set_id=6)
        )

        loop_cm = tc.For_i(0, loop_M, 1) if loop_M else None
        if loop_cm is not None:
            ctx.enter_context(loop_cm)

        # targets: one strided SWDGE DMA (separate queue from the x stream).
        # Tile is f32r so it can be the x-bucket lhsT; ttf = f32 view for DVE.
        ttr = small.tile([P, G], F32R, tag="ttr")
        t_src = bass.AP(tensor=t_d, offset=0, ap=[[1, P], [P, G]])
        nc.gpsimd.dma_start(out=ttr, in_=t_src)
        ttf = ttr.bitcast(F32)

        AH = small.tile([P, G], F32, tag="AH")      # per-row A' accumulators
        SV = small.tile([P, G], F32, tag="SV")      # per-row sum of m (ACT accum)
        SUX = psum.tile([1, 512], F32, tag="SUX")   # t-weighted sum of x

        n_x_mm = 0
        for g in range(G):
            # ---- load x (f32r tile; xf = f32 view for ACT/DVE) ----
            xt = xpool.tile([P, N], F32R, tag="xt")
            xf = xt.bitcast(F32)
            if g == 0 and split_first_dma:
                H = N // 2
                nc.sync.dma_start(out=xt[:, 0:H], in_=x[g][:, 0:H])
                nc.sync.dma_start(out=xt[:, H:N], in_=x[g][:, H:N])
            else:
                nc.sync.dma_start(out=xt, in_=x[g])

            # ---- e = exp(-x) ----
            if g in offload:
                # DVE exp2 bit trick: i16 = round(C1*x + C2) -> fp16 bits
                iet = epool.tile([P, N], I16, tag="iet")
                nc.vector.tensor_scalar(iet, xf, C1, C2, mult, add)
                et = iet.bitcast(F16)
            else:
                et = epool.tile([P, N], F16, tag="et")
                nc.scalar.activation(out=et, in_=xf, func=Exp, scale=-1.0)

            # ---- m = ln(1 + e) = softplus(-x); accum -> SV[:,g] = sum_j m ----
            mt = mpool.tile([P, N], F16, tag="mt")
            nc.scalar.activation(
                out=mt, in_=et, func=Ln, bias=1.0, accum_out=SV[:, g : g + 1]
            )

            # ---- sc = (m[1:] + K) + m[:-1]; sc[N-1] = K (sigma=0 -> u=0) ----
            sct = scpool.tile([P, N], F16, tag="sct")
            nc.vector.memset(sct[:, N - 1 : N], K)
            nc.vector.scalar_tensor_tensor(
                out=sct[:, 0 : N - 1],
                in0=mt[:, 1:N],
                scalar=K,
                in1=mt[:, 0 : N - 1],
                op0=add,
                op1=add,
            )

            # ---- i16 = int16(max(sc*C1, 0)); w = bits as fp16 ----
            i16t = ipool.tile([P, N], I16, tag="i16t")
            nc.vector.tensor_scalar(i16t, sct, C1, 0.0, mult, amax)
            wt = i16t.bitcast(F16)

            # ---- u = (sc - K) * w = sigma*exp(-sigma); A'[:,g] = sum_j u ----
            ut = upool.tile([P, N], F16, tag="ut")
            nc.vector.scalar_tensor_tensor(
                out=ut,
                in0=sct,
                scalar=-K,
                in1=wt,
                op0=add,
                op1=mult,
                accum_out=AH[:, g : g + 1],
            )

            # ---- PE bucket: SUX += t^T x (single stream, one lhsT/group) ----
            for c in range(CH):
                nc.tensor.matmul(
                    SUX,
                    ttr[:, g : g + 1],
                    xt[:, c * 512 : (c + 1) * 512],
                    start=(n_x_mm == 0),
                    stop=(n_x_mm == G * CH - 1),
                )
                n_x_mm += 1

        # ---- outputs: loss_rows = t*(A' + 2*SV/N), su[1] = x-bucket ----
        c0 = small.tile([P, G], F32, tag="c0")
        nc.vector.tensor_scalar(c0, SV, 2.0 / N, None, mult)
        nc.vector.tensor_add(c0, c0, AH)
        L = small.tile([P, G], F32, tag="L")
        nc.vector.tensor_mul(L, c0, ttf)
        nc.sync.dma_start(out=out_d.ap(), in_=L)

        susx = small.tile([1, 512], F32, tag="susx")
        nc.vector.tensor_copy(susx, SUX)
        nc.sync.dma_start(out=su_d.ap(), in_=susx)

    nc.finalize()
    return nc


_NC_CACHE = {}

# Groups whose exp(-x) runs on DVE instead of ACT (balance the engines).
BEST_OFFLOAD = ()


def _get_nc():
    if "nc" not in _NC_CACHE:
        _NC_CACHE["nc"] = build_kernel(offload=BEST_OFFLOAD)
    return _NC_CACHE["nc"]


def run_sharded(inputs, targets, trace=False, nc=None):
    if nc is None:
        nc = _get_nc()
    in_maps = []
    for c in range(NCORES):
        xs = np.ascontiguousarray(
            inputs[c * ROWS : (c + 1) * ROWS].reshape(G, P, N), dtype=np.float32
        )
        ts = np.ascontiguousarray(
            targets[c * ROWS : (c + 1) * ROWS].reshape(G, P, 1), dtype=np.float32
        )
        in_maps.append({"x": xs, "t": ts})
    res = run_bass_kernel_spmd(
        nc, in_maps, core_ids=list(range(NCORES)), trace=trace
    )
    total = 0.0
    for r in res.results:
        total += r["loss_rows"].astype(np.float64).sum()
        total += r["su"].astype(np.float64).sum() / N
    loss = np.float32(total / B)
    return loss, res


def kernel(inputs, targets):
    inputs = np.asarray(inputs, dtype=np.float32)
    targets = np.asarray(targets, dtype=np.float32)
    loss, _ = run_sharded(inputs, targets, trace=False)
    return loss
